# revision 1
# baseline (speedup 1.0000x reference)
"""Trainium2 Bass kernel for causal multi-head attention (B=4, T=2048, D=1024, H=16).

Sharding: tensor-parallel over heads. Each of the 8 NeuronCores owns 2 heads:
it computes Q/K/V projections for its head-slice over all tokens, runs causal
attention, then an AllToAll re-shards the attention output from head-sharded
to token-sharded so each core computes the final FC layer for its 1024-token
block with the full weight matrix. No reduction collective is needed.

All matmuls run as float32r (TF32-like, full PE rate at moving dim >= 256);
everything else stays fp32. Scores are computed transposed (S^T = K Q^T) so
softmax normalization lands on the PV matmul's free dim; the denominator is
obtained by augmenting V with a ones column, and its reciprocal is broadcast
across partitions with a selector matmul (partition-shifted DVE/DMA ops are
avoided entirely -- they were observed to misbehave on this stack).
"""
import sys

for _p in ("/opt/trn_rl_repo",):
    if _p not in sys.path:
        sys.path.insert(0, _p)

import numpy as np

import concourse.bass as bass
import concourse.mybir as mybir
import concourse.tile as tile
from concourse import bacc
from concourse.bass_utils import run_bass_kernel_spmd

f32 = mybir.dt.float32
f32r = mybir.dt.float32r
bf16 = mybir.dt.bfloat16
EXP = mybir.ActivationFunctionType.Exp

B, T, D, H, HD = 4, 2048, 1024, 16, 64
NCORES = 8
HPC = H // NCORES          # heads per core = 2
BT = B * T                 # 8192
CH = 512                   # token chunk (matmul moving dim)
NCH_B = T // CH            # 4 projection chunks per batch
QC = T // CH               # 4 query chunks per batch
NKV_B = T // 128           # 16 kv tiles of 128 per batch
ROWS = BT // NCORES        # 1024 output token rows per core
SCALE = 1.0 / 8.0          # 1/sqrt(HD)

_CACHE = {}


def _build(sim=False, no_collective=False, reps=1, n_ag=1, phases=('qkv','attn','fc'), dup=()):
    nc = bacc.Bacc("TRN2", target_bir_lowering=False, debug=False,
                   num_devices=1 if sim else NCORES)

    xT = nc.dram_tensor("xT", [D, BT], bf16, kind="ExternalInput").ap()
    wqkv = nc.dram_tensor("wqkv", [D, 3 * 128], bf16, kind="ExternalInput").ap()
    bqkv = nc.dram_tensor("bqkv", [1, 3 * 128], f32, kind="ExternalInput").ap()
    wfc_d = nc.dram_tensor("wfc", [D, 128], f32, kind="ExternalInput").ap()
    bfc_d = nc.dram_tensor("bfc", [1, 128], f32, kind="ExternalInput").ap()
    hm_d = nc.dram_tensor("hm", [128, 896], f32, kind="ExternalInput").ap()
    zl_d = nc.dram_tensor("zl", [65, 64], f32, kind="ExternalInput").ap()
    id_d = nc.dram_tensor("ident", [128, 128], f32, kind="ExternalInput").ap()
    ones_d = nc.dram_tensor("ones", [128, CH], f32, kind="ExternalInput").ap()
    zer_d = nc.dram_tensor("zer", [65, CH], f32, kind="ExternalInput").ap()
    outT = nc.dram_tensor("outT", [128, BT], f32, kind="ExternalOutput").ap()

    with tile.TileContext(nc) as tc:
        with tc.tile_pool(name="const", bufs=1) as cst, \
             tc.tile_pool(name="dram", bufs=1, space="DRAM") as dpool:

            # ---- constants (host-provided) ----
            ones_r = cst.tile([128, CH], f32r)
            nc.sync.dma_start(ones_r[:], ones_d[:].bitcast(f32r))
            hm = cst.tile([128, 896], f32)
            nc.sync.dma_start(hm[:], hm_d[:])
            zl = cst.tile([65, 64], f32r)       # selector: row 64 = 1, rest 0
            nc.sync.dma_start(zl[:], zl_d[:].bitcast(f32r))
            bias_q = cst.tile([1, 3 * 128], f32r)
            nc.sync.dma_start(bias_q[:], bqkv[:].bitcast(f32r))
            bias_f = cst.tile([1, 128], f32r)
            nc.sync.dma_start(bias_f[:], bfc_d[:].bitcast(f32r))
            # reciprocal staging tiles (rows 0..63 stay zero forever)
            rc = []
            for h in range(HPC):
                t = cst.tile([65, CH], f32r, name=f"recip{h}")
                nc.sync.dma_start(t[:], zer_d[:].bitcast(f32r))
                rc.append(t)
            rtmp = cst.tile([65, CH], f32)      # fp32 reciprocal staging
            ones_b = cst.tile([1, CH], bf16)
            nc.vector.tensor_copy(ones_b[:], ones_r[0:1, :].bitcast(f32))
            biasb = cst.tile([1, 3 * 128], bf16)
            nc.vector.tensor_copy(biasb[:], bias_q[:].bitcast(f32))

            # attention output, head-sharded: per local head [64, BT]
            attn = [cst.tile([64, BT], f32r, name=f"attn{h}")
                    for h in range(HPC)]

            # qkv weights: 8 d-tiles of [128, 384] = [q128 | k128 | v128]
            wq = cst.tile([128, 8 * 384], bf16)
            for d in range(8):
                nc.sync.dma_start(wq[:, d * 384:(d + 1) * 384],
                                  wqkv[d * 128:(d + 1) * 128, :])

            for _rep in range(reps):
                # ---- per-batch QKV projection + attention ----
                with tc.tile_pool(name="work", bufs=1) as wk, \
                     tc.tile_pool(name="ps", bufs=1, space="PSUM") as ps:
                    for b in range(B):
                        t0 = b * T
                        qt = wk.tile([128, T], f32r, tag="qt", bufs=2, name=f"qt{b}")
                        kt = wk.tile([128, T], f32r, tag="kt", bufs=2, name=f"kt{b}")
                        vsb = wk.tile([128, NKV_B * 130], f32r, tag="vsb", bufs=2,
                                      name=f"vsb{b}")
                        # ones columns (denominator) for all 16 kv tiles of batch b
                        v3 = vsb.rearrange("p (t c) -> p t c", c=130)
                        src1 = ones_d[:, 0:NKV_B].rearrange("p (t c) -> p t c", c=1)
                        nc.sync.dma_start(v3[:, :, 64:65], src1.bitcast(f32r))
                        nc.sync.dma_start(v3[:, :, 129:130], src1.bitcast(f32r))

                        for ch in [c for c in range(NCH_B) for _ in range(2 if 'qkv' in dup else 1)]:
                            c0 = t0 + ch * CH
                            xt = wk.tile([128, 8 * CH], bf16, tag="xt", bufs=2,
                                         name=f"xt{b}_{ch}")
                            xt3 = xt.rearrange("p (d c) -> p d c", d=8)
                            xs3 = xT[:, c0:c0 + CH].rearrange(
                                "(d p) c -> p d c", p=128)
                            nc.sync.dma_start(xt3[:], xs3)
                            cs = ch * CH
                            # Q^T chunk
                            psq = ps.tile([128, CH], f32, tag="mm", bufs=2,
                                          name=f"psq{b}_{ch}")
                            for d in range(8):
                                nc.tensor.matmul(psq[:],
                                                 wq[:, d * 384:d * 384 + 128],
                                                 xt[:, d * CH:(d + 1) * CH],
                                                 start=(d == 0), stop=False)
                            nc.tensor.matmul(psq[:], bias_q[0:1, 0:128],
                                             ones_r[0:1, :], start=False, stop=True)
                            nc.vector.tensor_copy(qt[:, cs:cs + CH], psq[:])
                            # K^T chunk
                            psk = ps.tile([128, CH], f32, tag="mm", bufs=2,
                                          name=f"psk{b}_{ch}")
                            for d in range(8):
                                nc.tensor.matmul(psk[:],
                                                 wq[:, d * 384 + 128:d * 384 + 256],
                                                 xt[:, d * CH:(d + 1) * CH],
                                                 start=(d == 0), stop=False)
                            nc.tensor.matmul(psk[:], bias_q[0:1, 128:256],
                                             ones_r[0:1, :], start=False, stop=True)
                            nc.vector.tensor_copy(kt[:, cs:cs + CH], psk[:])
                            # V directly token-major: lhsT = x tile, rhs = W_v
                            for sb in range(CH // 128):
                                kvt = ch * 4 + sb   # kv tile idx within batch
                                psv = ps.tile([128, 128], f32, tag="mm", bufs=2,
                                              name=f"psv{b}_{ch}_{sb}")
                                for d in range(8):
                                    nc.tensor.matmul(
                                        psv[:],
                                        xt3[:, d, sb * 128:(sb + 1) * 128],
                                        wq[:, d * 384 + 256:d * 384 + 384],
                                        start=(d == 0), stop=False)
                                nc.tensor.matmul(psv[:], ones_b[0:1, 0:128],
                                                 biasb[0:1, 256:384],
                                                 start=False, stop=True)
                                base = kvt * 130
                                nc.vector.tensor_copy(vsb[:, base:base + 64],
                                                      psv[:, 0:64])
                                nc.vector.tensor_copy(vsb[:, base + 65:base + 129],
                                                      psv[:, 64:128])

                        # ---- causal attention for batch b ----
                        for qc in range(QC):
                            g0 = t0 + qc * CH
                            nkv = 4 * (qc + 1)
                            pv = [ps.tile([128, CH], f32, tag=f"pv{h}", bufs=1,
                                          name=f"pv{h}_{b}_{qc}")
                                  for h in range(HPC)]
                            for ki in range(nkv):
                                diag = ki - 4 * qc  # >=0 on diagonal block tiles
                                st = ps.tile([128, 2 * CH], f32, tag="s", bufs=2,
                                             name=f"s_{b}_{qc}_{ki}")
                                pt = wk.tile([128, 2 * CH], f32r, tag="p", bufs=3,
                                             name=f"p_{b}_{qc}_{ki}")
                                for h in range(HPC):
                                    nc.tensor.matmul(
                                        st[:, h * CH:(h + 1) * CH],
                                        kt[64 * h:64 * h + 64,
                                           ki * 128:(ki + 1) * 128],
                                        qt[64 * h:64 * h + 64,
                                           qc * CH:(qc + 1) * CH],
                                        start=True, stop=True,
                                        tile_position=(64 * h, 0))
                                nc.scalar.activation(pt[:], st[:], EXP,
                                                     scale=SCALE)
                                if diag >= 0:
                                    off = 384 - 128 * diag
                                    for h in range(HPC):
                                        nc.vector.tensor_mul(
                                            pt[:, h * CH:(h + 1) * CH],
                                            pt[:, h * CH:(h + 1) * CH],
                                            hm[:, off:off + CH])
                                for h in range(HPC):
                                    vb = ki * 130 + 65 * h
                                    nc.tensor.matmul(pv[h][0:65, :],
                                                     vsb[:, vb:vb + 65],
                                                     pt[:, h * CH:(h + 1) * CH],
                                                     start=(ki == 0),
                                                     stop=(ki == nkv - 1))
                            # normalize: reciprocal of denom row, broadcast, mul
                            for h in range(HPC):
                                nc.vector.reciprocal(rtmp[64:65, :],
                                                     pv[h][64:65, :])
                                nc.vector.tensor_copy(rc[h][64:65, :],
                                                      rtmp[64:65, :])
                                bc = ps.tile([64, CH], f32, tag="mm", bufs=2,
                                             name=f"bc{h}_{b}_{qc}")
                                nc.tensor.matmul(bc[:], zl[:], rc[h][:],
                                                 start=True, stop=True)
                                rb = wk.tile([64, CH], f32, tag="rb", bufs=2,
                                             name=f"rb{h}_{b}_{qc}")
                                nc.vector.tensor_copy(rb[:], bc[:])
                                nc.vector.tensor_mul(attn[h][:, g0:g0 + CH],
                                                     pv[h][0:64, :], rb[:])

                # ---- per-batch AllGather (overlaps later batches) ----
                ag_outs = []
                for b in range(B):
                    t0 = b * T
                    ag_in = dpool.tile([128, T], f32, name=f"ag_in{b}")
                    ag_out = dpool.tile([NCORES * 128, T], f32,
                                        name=f"ag_out{b}")
                    for h in range(HPC):
                        nc.sync.dma_start(
                            ag_in[64 * h:64 * h + 64, :],
                            attn[h][:, t0:t0 + T].bitcast(f32))
                    if sim or no_collective:
                        nc.sync.dma_start(ag_out[0:128, :], ag_in[:])
                    else:
                        for _agi in range(n_ag):
                            nc.gpsimd.collective_compute(
                                "AllGather", mybir.AluOpType.bypass,
                                replica_groups=[list(range(NCORES))],
                                ins=[ag_in.opt()], outs=[ag_out.opt()])
                    ag_outs.append(ag_out)

                # ---- final FC: this core computes its 128 output features for
                # all tokens (weight slice is per-core host input) ----
                with tc.tile_pool(name="fcp", bufs=1) as fcp, \
                     tc.tile_pool(name="psc", bufs=1, space="PSUM") as psc:
                    wfc = fcp.tile([128, 8 * 128], f32r)
                    for d in range(8):
                        nc.sync.dma_start(
                            wfc[:, d * 128:(d + 1) * 128],
                            wfc_d[d * 128:(d + 1) * 128, :].bitcast(f32r))
                    for oc in [o for o in range(BT // CH if 'fc' in phases else 0) for _ in range(2 if 'fc' in dup else 1)]:
                        fci = fcp.tile([128, 8 * CH], f32r, tag="fci", bufs=3,
                                       name=f"fci{oc}")
                        agb = ag_outs[oc // QC]
                        lc = oc % QC
                        for d in range(8):
                            nc.sync.dma_start(
                                fci[:, d * CH:(d + 1) * CH],
                                agb[d * 128:(d + 1) * 128,
                                    lc * CH:(lc + 1) * CH].bitcast(f32r))
                        pfc = psc.tile([128, CH], f32, tag="fc", bufs=4,
                                       name=f"pfc{oc}")
                        for d in range(8):
                            nc.tensor.matmul(pfc[:],
                                             wfc[:, d * 128:(d + 1) * 128],
                                             fci[:, d * CH:(d + 1) * CH],
                                             start=(d == 0), stop=False)
                        nc.tensor.matmul(pfc[:], bias_f[0:1, :],
                                         ones_r[0:1, :], start=False, stop=True)
                        ost = fcp.tile([128, CH], f32, tag="ost", bufs=4,
                                       name=f"ost{oc}")
                        nc.vector.tensor_copy(ost[:], pfc[:])
                        nc.sync.dma_start(outT[:, oc * CH:(oc + 1) * CH], ost[:])

    nc.compile()
    return nc


def _host_inputs(x, W_qkv, b_qkv, W_fc, b_fc):
    import ml_dtypes
    x = np.asarray(x, dtype=np.float32)
    W_qkv = np.asarray(W_qkv, dtype=np.float32)
    b_qkv = np.asarray(b_qkv, dtype=np.float32)
    W_fc = np.asarray(W_fc, dtype=np.float32)
    b_fc = np.asarray(b_fc, dtype=np.float32)

    xT = np.ascontiguousarray(x.reshape(BT, D).T).astype(ml_dtypes.bfloat16)
    hm = (np.arange(128)[:, None]
          <= np.arange(896)[None, :] - 384).astype(np.float32)
    zl = np.zeros((65, 64), np.float32)
    zl[64, :] = 1.0
    ident = np.eye(128, dtype=np.float32)
    ones = np.ones((128, CH), np.float32)
    zer = np.zeros((65, CH), np.float32)
    in_maps = []
    for c in range(NCORES):
        f0 = c * (HPC * HD)  # 128*c
        wfc_c = np.ascontiguousarray(W_fc[:, f0:f0 + 128])
        bfc_c = np.ascontiguousarray(b_fc[None, f0:f0 + 128])
        wq_c = np.ascontiguousarray(np.concatenate(
            [W_qkv[:, p * D + f0: p * D + f0 + 128] for p in range(3)],
            axis=1).astype(ml_dtypes.bfloat16))
        bq_c = np.ascontiguousarray(np.concatenate(
            [b_qkv[p * D + f0: p * D + f0 + 128] for p in range(3)])[None, :])
        in_maps.append({
            "xT": xT, "wqkv": wq_c, "bqkv": bq_c, "wfc": wfc_c, "bfc": bfc_c,
            "hm": hm, "zl": zl, "ident": ident, "ones": ones, "zer": zer,
        })
    return in_maps


def _get_nc():
    if "nc" not in _CACHE:
        _CACHE["nc"] = _build()
    return _CACHE["nc"]


def _assemble(results):
    blocks = [results[c]["outT"] for c in range(NCORES)]
    full = np.concatenate(blocks, axis=0)          # [D, BT], feature-major
    return np.ascontiguousarray(full.T).reshape(B, T, D).astype(np.float32)


def kernel(x, W_qkv, b_qkv, W_fc, b_fc):
    nc = _get_nc()
    in_maps = _host_inputs(x, W_qkv, b_qkv, W_fc, b_fc)
    res = run_bass_kernel_spmd(nc, in_maps, list(range(NCORES)))
    return _assemble(res.results)



# revision 41
# speedup vs baseline: 13274.1453x; 13274.1453x over previous
"""Trainium2 Bass kernel for causal multi-head attention (B=4, T=2048, D=1024, H=16).

Sharding: tensor-parallel over heads. Each of the 8 NeuronCores owns 2 heads:
it computes Q/K/V projections for its head-slice over all tokens, runs causal
attention, then an AllGather re-shards the attention output from head-sharded
to token-major-full so each core computes the final FC layer for its 128
output features over all tokens. No reduction collective is needed.

v7 structure: software-pipelined at two levels. Across batches, batch b's
attention runs while batch b+1's QKV projection and batch b-1's FC layer
execute -- their matmuls are interleaved as dependency-free PE "filler"
between the P@V matmuls of the attention inner loop, so the PE never idles
waiting out the score -> exp -> mask chain (the attention-layer dependency
latency is ~1.5us per kv tile but its PE work is only ~0.85us). Within the
inner loop, scores run one kv tile ahead of P@V.

Numerics: Q/K/V, P = exp(S), the attention output, AllGather payload, and
FC input are bf16; matmul accumulation is fp32 in PSUM. The K projection
bias is dropped (it shifts every score in a softmax row by a constant) and
the V bias is folded into the FC bias on the host (b_fc' = b_fc + b_v@W_fc).
Scores are computed transposed (S^T = K Q^T) so softmax normalization lands
on the PV matmul's free dim; the denominator comes from a ones column
appended to V, and its reciprocal is broadcast across partitions with a
K=1 ones-row matmul (partition-shifted DVE/DMA ops misbehave on this stack).
Q/FC biases are added by the DVE during the PSUM drain (per-partition
tensor_scalar); diagonal score matmuls skip fully-masked columns (the causal
mask multiply cleans the stale columns, zeroed once at startup).
"""
import sys
import itertools

for _p in ("/opt/trn_rl_repo",):
    if _p not in sys.path:
        sys.path.insert(0, _p)

import numpy as np

import concourse.bass as bass
import concourse.mybir as mybir
import concourse.tile as tile
from concourse import bacc
from concourse.bass_utils import run_bass_kernel_spmd

f32 = mybir.dt.float32
f32r = mybir.dt.float32r
bf16 = mybir.dt.bfloat16
EXP = mybir.ActivationFunctionType.Exp

B, T, D, H, HD = 4, 2048, 1024, 16, 64
NCORES = 8
HPC = H // NCORES          # heads per core = 2
BT = B * T                 # 8192
CH = 512                   # token chunk (matmul moving dim)
NCH_B = T // CH            # 4 projection chunks per batch
QC = T // CH               # 4 query chunks per batch
NKV_B = T // 128           # 16 kv tiles of 128 per batch
SCALE = 1.0 / 8.0          # 1/sqrt(HD)

_CACHE = {}


def _build(sim=False, no_collective=False, reps=1, n_ag=1, strip=6,
           fill_per_ki=3):
    # strip: timing-diagnostic ladder. 6=full kernel; 5=drop ag/fc;
    # 4=also drop normalize; 3=also drop pv; 2=also drop causal mask;
    # 1=also drop exp; 0=qkv only.
    nc = bacc.Bacc("TRN2", target_bir_lowering=False, debug=False,
                   num_devices=1 if sim else NCORES)

    xT = nc.dram_tensor("xT", [D, BT], bf16, kind="ExternalInput").ap()
    wqkv = nc.dram_tensor("wqkv", [D, 3 * 128], bf16, kind="ExternalInput").ap()
    bq_d = nc.dram_tensor("bq", [128, 1], f32, kind="ExternalInput").ap()
    wfc_d = nc.dram_tensor("wfc", [D, 128], bf16, kind="ExternalInput").ap()
    bfc_d = nc.dram_tensor("bfc", [128, 1], f32, kind="ExternalInput").ap()
    hm_d = nc.dram_tensor("hm", [128, 896], bf16, kind="ExternalInput").ap()
    ones_d = nc.dram_tensor("ones", [128, CH], f32, kind="ExternalInput").ap()
    onesh_d = nc.dram_tensor("onesh", [128, 64], bf16, kind="ExternalInput").ap()
    outT = nc.dram_tensor("outT", [128, BT], f32, kind="ExternalOutput").ap()

    with tile.TileContext(nc) as tc:
        with tc.tile_pool(name="const", bufs=1) as cst, \
             tc.tile_pool(name="dram", bufs=1, space="DRAM") as dpool, \
             tc.tile_pool(name="work", bufs=1) as wk, \
             tc.tile_pool(name="ps", bufs=1, space="PSUM") as ps:

            # ---- constants (host-provided) ----
            ones_r = cst.tile([1, CH], f32r)
            nc.sync.dma_start(ones_r[:], ones_d[0:1, :].bitcast(f32r))
            onesh = cst.tile([128, 64], bf16)
            nc.sync.dma_start(onesh[:], onesh_d[:])
            hm = cst.tile([128, 896], bf16)
            nc.sync.dma_start(hm[:], hm_d[:])
            bias_q = cst.tile([128, 1], f32)     # per-partition Q bias
            nc.sync.dma_start(bias_q[:], bq_d[:])
            bias_f = cst.tile([128, 1], f32)     # per-partition FC bias
            nc.sync.dma_start(bias_f[:], bfc_d[:])
            rc = [cst.tile([1, CH], bf16, name=f"recip{h}")
                  for h in range(HPC)]

            # qkv weights: 8 d-tiles of [128, 384] = [q128 | k128 | v128]
            wq = cst.tile([128, 8 * 384], bf16)
            for d in range(8):
                nc.sync.dma_start(wq[:, d * 384:(d + 1) * 384],
                                  wqkv[d * 128:(d + 1) * 128, :])
            # fc weights: 8 d-tiles of [128, 128]
            wfc = cst.tile([128, 8 * 128], bf16)
            for d in range(8):
                nc.sync.dma_start(wfc[:, d * 128:(d + 1) * 128],
                                  wfc_d[d * 128:(d + 1) * 128, :])

            # zero the rotating score-PSUM buffers once: diagonal score
            # matmuls leave stale left columns that exp reads before the
            # mask zeroes them -- first use must not see NaN bit patterns
            for zi in range(2):
                stz = ps.tile([128, 2 * CH], f32, tag="s", bufs=2,
                              name=f"stz{zi}")
                nc.scalar.memzero(stz[:])

            def make_tiles(b, rep):
                qt = wk.tile([128, T], bf16, tag="qt", bufs=2,
                             name=f"qt{b}_{rep}")
                kt = wk.tile([128, T], bf16, tag="kt", bufs=2,
                             name=f"kt{b}_{rep}")
                vsb = wk.tile([128, NKV_B * 130], bf16, tag="vsb", bufs=2,
                              name=f"vsb{b}_{rep}")
                # ones columns (softmax denominator) for all 16 kv tiles
                v3 = vsb.rearrange("p (t c) -> p t c", c=130)
                src1 = onesh[:, 0:NKV_B].rearrange("p (t c) -> p t c", c=1)
                nc.sync.dma_start(v3[:, :, 64:65], src1)
                nc.sync.dma_start(v3[:, :, 129:130], src1)
                return qt, kt, vsb

            def qkv_steps(b, rep, tiles):
                """Generator: one yield per PE step of batch b's projection."""
                qt, kt, vsb = tiles
                xts = {}

                def xt_dma(ch):
                    xt = wk.tile([128, 8 * CH], bf16, tag="xt", bufs=2,
                                 name=f"xt{b}_{ch}_{rep}")
                    nc.sync.dma_start(
                        xt.rearrange("p (d c) -> p d c", d=8),
                        xT[:, b * T + ch * CH:b * T + (ch + 1) * CH]
                        .rearrange("(d p) c -> p d c", p=128))
                    xts[ch] = xt

                xt_dma(0)
                for ch in range(NCH_B):
                    if ch + 1 < NCH_B:
                        xt_dma(ch + 1)
                    xt = xts.pop(ch)
                    xt3 = xt.rearrange("p (d c) -> p d c", d=8)
                    cs = ch * CH
                    # Q^T chunk (bias added per-partition on PSUM drain)
                    psq = ps.tile([128, CH], f32, tag="mm", bufs=2,
                                  name=f"psq{b}_{ch}_{rep}")
                    for d in range(8):
                        nc.tensor.matmul(psq[:], wq[:, d * 384:d * 384 + 128],
                                         xt[:, d * CH:(d + 1) * CH],
                                         start=(d == 0), stop=(d == 7))
                        yield
                    nc.vector.tensor_scalar_add(qt[:, cs:cs + CH], psq[:],
                                                bias_q[:])
                    # K^T chunk (bias dropped: softmax-invariant)
                    psk = ps.tile([128, CH], f32, tag="mm", bufs=2,
                                  name=f"psk{b}_{ch}_{rep}")
                    for d in range(8):
                        nc.tensor.matmul(psk[:],
                                         wq[:, d * 384 + 128:d * 384 + 256],
                                         xt[:, d * CH:(d + 1) * CH],
                                         start=(d == 0), stop=(d == 7))
                        yield
                    nc.vector.tensor_copy(kt[:, cs:cs + CH], psk[:])
                    # V token-major (bias folded into FC bias on host)
                    for sb in range(CH // 128):
                        kvt = ch * 4 + sb
                        psv = ps.tile([128, 128], f32, tag="mm", bufs=2,
                                      name=f"psv{b}_{ch}_{sb}_{rep}")
                        for d in range(8):
                            nc.tensor.matmul(
                                psv[:], xt3[:, d, sb * 128:(sb + 1) * 128],
                                wq[:, d * 384 + 256:d * 384 + 384],
                                start=(d == 0), stop=(d == 7))
                        yield
                        # both heads' 64 V columns in one strided copy,
                        # skipping the interleaved ones columns
                        v4 = vsb.rearrange("p (t g c) -> p t g c", g=2, c=65)
                        nc.vector.tensor_copy(
                            v4[:, kvt, :, 0:64],
                            psv.rearrange("p (g c) -> p g c", c=64))

            def fc_steps(b, rep, ag_out):
                """Generator: one yield per PE step of batch b's FC layer."""
                ost = wk.tile([128, T], f32, tag="ost", bufs=2,
                              name=f"ost{b}_{rep}")
                fcis = {}

                def fci_dma(lc):
                    fci = wk.tile([128, 8 * CH], bf16, tag="fci", bufs=3,
                                  name=f"fci{b}_{lc}_{rep}")
                    nc.scalar.dma_start(
                        fci.rearrange("p (d c) -> p d c", d=8),
                        ag_out[:, lc * CH:(lc + 1) * CH]
                        .rearrange("(d p) c -> p d c", p=128))
                    fcis[lc] = fci

                fci_dma(0)
                for lc in range(QC):
                    if lc + 1 < QC:
                        fci_dma(lc + 1)
                    fci = fcis.pop(lc)
                    oc = b * QC + lc
                    pfc = ps.tile([128, CH], f32, tag="mm", bufs=2,
                                  name=f"pfc{oc}_{rep}")
                    for d in range(8):
                        nc.tensor.matmul(pfc[:], wfc[:, d * 128:(d + 1) * 128],
                                         fci[:, d * CH:(d + 1) * CH],
                                         start=(d == 0), stop=(d == 7))
                        yield
                    nc.vector.tensor_scalar_add(ost[:, lc * CH:(lc + 1) * CH],
                                                pfc[:], bias_f[:])
                nc.sync.dma_start(outT[:, b * T:(b + 1) * T], ost[:])

            def attn_batch(b, rep, tiles, filler):
                """Causal attention for batch b; pulls filler PE steps
                between P@V matmuls to hide the score->exp->mask latency."""
                qt, kt, vsb = tiles
                attn_b = wk.tile([128, T], bf16, tag="attn", bufs=2,
                                 name=f"attn{b}_{rep}") if strip >= 5 else None

                def pull(n):
                    for _ in range(n):
                        if next(filler, None) is None:
                            break

                for qc in range(QC):
                    nkv = 4 * (qc + 1)
                    pv = [ps.tile([128, CH], f32, tag=f"pv{h}", bufs=1,
                                  name=f"pv{h}_{b}_{qc}_{rep}")
                          for h in range(HPC)] if strip >= 4 else None

                    def scores(ki):
                        diag = ki - 4 * qc  # >=0 on diagonal block tiles
                        lo = 128 * diag if diag > 0 else 0
                        st = ps.tile([128, 2 * CH], f32, tag="s", bufs=2,
                                     name=f"s_{b}_{qc}_{ki}_{rep}")
                        for h in range(HPC):
                            nc.tensor.matmul(
                                st[:, h * CH + lo:(h + 1) * CH],
                                kt[64 * h:64 * h + 64,
                                   ki * 128:(ki + 1) * 128],
                                qt[64 * h:64 * h + 64,
                                   qc * CH + lo:(qc + 1) * CH],
                                start=True, stop=True,
                                tile_position=(64 * h, 0))
                        if strip < 2:
                            return None
                        pt = wk.tile([128, 2 * CH], bf16, tag="p", bufs=4,
                                     name=f"p_{b}_{qc}_{ki}_{rep}")
                        nc.scalar.activation(pt[:], st[:], EXP, scale=SCALE)
                        if diag >= 0 and strip >= 3:
                            off = 384 - 128 * diag
                            for h in range(HPC):
                                nc.vector.tensor_mul(
                                    pt[:, h * CH:(h + 1) * CH],
                                    pt[:, h * CH:(h + 1) * CH],
                                    hm[:, off:off + CH])
                        return pt

                    # scores run one kv tile ahead of P@V
                    pts = {0: scores(0)}
                    for ki in range(nkv):
                        if ki + 1 < nkv:
                            pts[ki + 1] = scores(ki + 1)
                        pull(fill_per_ki)
                        pt = pts.pop(ki)
                        if strip < 4:
                            continue
                        for h in range(HPC):
                            vb = ki * 130 + 65 * h
                            nc.tensor.matmul(pv[h][0:65, :],
                                             vsb[:, vb:vb + 65],
                                             pt[:, h * CH:(h + 1) * CH],
                                             start=(ki == 0),
                                             stop=(ki == nkv - 1))
                    if strip < 5:
                        continue
                    # normalize: reciprocal of denom row, broadcast via K=1
                    # ones-row matmul, multiply into attn_b (bf16); the two
                    # heads' chains are interleaved so they pipeline
                    with nc.allow_low_precision(
                            reason="softmax reciprocal broadcast in bf16; "
                                   "0.4% rounding is within tolerance"):
                        for h in range(HPC):
                            nc.vector.reciprocal(rc[h][:], pv[h][64:65, :])
                    rbs = []
                    for h in range(HPC):
                        bc = ps.tile([64, CH], f32, tag="mm", bufs=2,
                                     name=f"bc{h}_{b}_{qc}_{rep}")
                        nc.tensor.matmul(bc[:], onesh[0:1, 0:64], rc[h][:],
                                         start=True, stop=True)
                        rb = wk.tile([64, CH], f32, tag="rb", bufs=2,
                                     name=f"rb{h}_{b}_{qc}_{rep}")
                        nc.vector.tensor_copy(rb[:], bc[:])
                        rbs.append(rb)
                    for h in range(HPC):
                        nc.vector.tensor_mul(
                            attn_b[64 * h:64 * h + 64,
                                   qc * CH:(qc + 1) * CH],
                            pv[h][0:64, :], rbs[h][:])
                # drain whatever filler the attention loop did not consume
                pull(1 << 20)
                return attn_b

            def ag_batch(b, rep, attn_b):
                ag_in = dpool.tile([128, T], bf16, name=f"ag_in{b}_{rep}")
                ag_out = dpool.tile([NCORES * 128, T], bf16,
                                    name=f"ag_out{b}_{rep}",
                                    addr_space="Shared")
                nc.sync.dma_start(ag_in[:], attn_b[:])
                if sim or no_collective:
                    nc.sync.dma_start(ag_out[0:128, :], ag_in[:])
                else:
                    nc.gpsimd.collective_compute(
                        "AllGather", mybir.AluOpType.bypass,
                        replica_groups=[list(range(NCORES))],
                        ins=[ag_in.opt()], outs=[ag_out.opt()])
                    for _agi in range(1, n_ag):   # timing-ablation only
                        ag_x = dpool.tile([NCORES * 128, T], bf16,
                                          name=f"ag_x{b}_{rep}_{_agi}",
                                          addr_space="Shared")
                        nc.gpsimd.collective_compute(
                            "AllGather", mybir.AluOpType.bypass,
                            replica_groups=[list(range(NCORES))],
                            ins=[ag_in.opt()], outs=[ag_x.opt()])
                return ag_out

            for rep in range(reps):
                tiles = make_tiles(0, rep)
                for _ in qkv_steps(0, rep, tiles):
                    pass
                batch_tiles = {0: tiles}
                ag_outs = {}
                for b in range(B):
                    fill = []
                    if strip >= 1 and b + 1 < B:
                        batch_tiles[b + 1] = make_tiles(b + 1, rep)
                        fill.append(qkv_steps(b + 1, rep, batch_tiles[b + 1]))
                    if b - 1 in ag_outs:
                        fill.append(fc_steps(b - 1, rep, ag_outs.pop(b - 1)))
                    filler = itertools.chain(*fill)
                    if strip >= 1:
                        attn_b = attn_batch(b, rep, batch_tiles.pop(b), filler)
                    else:
                        attn_b = None
                    for _ in filler:   # force-drain leftover filler
                        pass
                    if strip >= 6:
                        ag_outs[b] = ag_batch(b, rep, attn_b)
                if B - 1 in ag_outs:
                    for _ in fc_steps(B - 1, rep, ag_outs.pop(B - 1)):
                        pass

    nc.compile()
    return nc


def _host_inputs(x, W_qkv, b_qkv, W_fc, b_fc):
    import ml_dtypes
    x = np.asarray(x, dtype=np.float32)
    W_qkv = np.asarray(W_qkv, dtype=np.float32)
    b_qkv = np.asarray(b_qkv, dtype=np.float32)
    W_fc = np.asarray(W_fc, dtype=np.float32)
    b_fc = np.asarray(b_fc, dtype=np.float32)

    xT = np.ascontiguousarray(x.reshape(BT, D).T).astype(ml_dtypes.bfloat16)
    hm = (np.arange(128)[:, None]
          <= np.arange(896)[None, :] - 384).astype(ml_dtypes.bfloat16)
    ones = np.ones((128, CH), np.float32)
    onesh = np.ones((128, 64), ml_dtypes.bfloat16)
    b_v = b_qkv[2 * D:3 * D]
    bfc_eff = b_fc + b_v @ W_fc      # V bias folded through the FC layer
    in_maps = []
    for c in range(NCORES):
        f0 = c * (HPC * HD)  # 128*c
        wfc_c = np.ascontiguousarray(W_fc[:, f0:f0 + 128]).astype(
            ml_dtypes.bfloat16)
        bfc_c = np.ascontiguousarray(bfc_eff[f0:f0 + 128, None])
        wq_c = np.ascontiguousarray(np.concatenate(
            [W_qkv[:, p * D + f0: p * D + f0 + 128] for p in range(3)],
            axis=1).astype(ml_dtypes.bfloat16))
        bq_c = np.ascontiguousarray(b_qkv[f0:f0 + 128, None])
        in_maps.append({
            "xT": xT, "wqkv": wq_c, "bq": bq_c, "wfc": wfc_c, "bfc": bfc_c,
            "hm": hm, "ones": ones, "onesh": onesh,
        })
    return in_maps


def _get_nc():
    if "nc" not in _CACHE:
        _CACHE["nc"] = _build()
    return _CACHE["nc"]


def _assemble(results):
    blocks = [results[c]["outT"] for c in range(NCORES)]
    full = np.concatenate(blocks, axis=0)          # [D, BT], feature-major
    return np.ascontiguousarray(full.T).reshape(B, T, D).astype(np.float32)


def kernel(x, W_qkv, b_qkv, W_fc, b_fc):
    nc = _get_nc()
    in_maps = _host_inputs(x, W_qkv, b_qkv, W_fc, b_fc)
    res = run_bass_kernel_spmd(nc, in_maps, list(range(NCORES)))
    return _assemble(res.results)


# revision 44
# speedup vs baseline: 13709.5779x; 1.0328x over previous
"""Trainium2 Bass kernel for causal multi-head attention (B=4, T=2048, D=1024, H=16).

Sharding: tensor-parallel over heads. Each of the 8 NeuronCores owns 2 heads:
it computes Q/K/V projections for its head-slice over all tokens, runs causal
attention, then an AllGather re-shards the attention output from head-sharded
to token-major-full so each core computes the final FC layer for its 128
output features over all tokens. No reduction collective is needed.

v7 structure: software-pipelined at two levels. Across batches, batch b's
attention runs while batch b+1's QKV projection and batch b-1's FC layer
execute -- their matmuls are interleaved as dependency-free PE "filler"
between the P@V matmuls of the attention inner loop, so the PE never idles
waiting out the score -> exp -> mask chain (the attention-layer dependency
latency is ~1.5us per kv tile but its PE work is only ~0.85us). Within the
inner loop, scores run one kv tile ahead of P@V.

Numerics: Q/K/V, P = exp(S), the attention output, AllGather payload, and
FC input are bf16; matmul accumulation is fp32 in PSUM. The K projection
bias is dropped (it shifts every score in a softmax row by a constant) and
the V bias is folded into the FC bias on the host (b_fc' = b_fc + b_v@W_fc).
Scores are computed transposed (S^T = K Q^T) so softmax normalization lands
on the PV matmul's free dim; the denominator comes from a ones column
appended to V, and its reciprocal is broadcast across partitions with a
K=1 ones-row matmul (partition-shifted DVE/DMA ops misbehave on this stack).
Q/FC biases are added by the DVE during the PSUM drain (per-partition
tensor_scalar); diagonal score matmuls skip fully-masked columns (the causal
mask multiply cleans the stale columns, zeroed once at startup).
"""
import sys
import itertools

for _p in ("/opt/trn_rl_repo",):
    if _p not in sys.path:
        sys.path.insert(0, _p)

import numpy as np

import concourse.bass as bass
import concourse.mybir as mybir
import concourse.tile as tile
from concourse import bacc
from concourse.bass_utils import run_bass_kernel_spmd

f32 = mybir.dt.float32
f32r = mybir.dt.float32r
bf16 = mybir.dt.bfloat16
EXP = mybir.ActivationFunctionType.Exp

B, T, D, H, HD = 4, 2048, 1024, 16, 64
NCORES = 8
HPC = H // NCORES          # heads per core = 2
BT = B * T                 # 8192
CH = 512                   # token chunk (matmul moving dim)
NCH_B = T // CH            # 4 projection chunks per batch
QC = T // CH               # 4 query chunks per batch
NKV_B = T // 128           # 16 kv tiles of 128 per batch
SCALE = 1.0 / 8.0          # 1/sqrt(HD)

_CACHE = {}


def _build(sim=False, no_collective=False, reps=1, n_ag=1, strip=6,
           fill_per_ki=3):
    # strip: timing-diagnostic ladder. 6=full kernel; 5=drop ag/fc;
    # 4=also drop normalize; 3=also drop pv; 2=also drop causal mask;
    # 1=also drop exp; 0=qkv only.
    nc = bacc.Bacc("TRN2", target_bir_lowering=False, debug=False,
                   num_devices=1 if sim else NCORES)

    xT = nc.dram_tensor("xT", [D, BT], bf16, kind="ExternalInput").ap()
    wqkv = nc.dram_tensor("wqkv", [D, 3 * 128], bf16, kind="ExternalInput").ap()
    bq_d = nc.dram_tensor("bq", [128, 1], f32, kind="ExternalInput").ap()
    wfc_d = nc.dram_tensor("wfc", [D, 128], bf16, kind="ExternalInput").ap()
    bfc_d = nc.dram_tensor("bfc", [128, 1], f32, kind="ExternalInput").ap()
    hm_d = nc.dram_tensor("hm", [128, 896], bf16, kind="ExternalInput").ap()
    ones_d = nc.dram_tensor("ones", [128, CH], f32, kind="ExternalInput").ap()
    onesh_d = nc.dram_tensor("onesh", [128, 64], bf16, kind="ExternalInput").ap()
    outT = nc.dram_tensor("outT", [128, BT], f32, kind="ExternalOutput").ap()

    with tile.TileContext(nc) as tc:
        with tc.tile_pool(name="const", bufs=1) as cst, \
             tc.tile_pool(name="dram", bufs=1, space="DRAM") as dpool, \
             tc.tile_pool(name="work", bufs=1) as wk, \
             tc.tile_pool(name="ps", bufs=1, space="PSUM") as ps:

            # ---- constants (host-provided) ----
            ones_r = cst.tile([1, CH], f32r)
            nc.sync.dma_start(ones_r[:], ones_d[0:1, :].bitcast(f32r))
            onesh = cst.tile([128, 64], bf16)
            nc.sync.dma_start(onesh[:], onesh_d[:])
            hm = cst.tile([128, 896], bf16)
            nc.sync.dma_start(hm[:], hm_d[:])
            bias_q = cst.tile([128, 1], f32)     # per-partition Q bias
            nc.sync.dma_start(bias_q[:], bq_d[:])
            bias_f = cst.tile([128, 1], f32)     # per-partition FC bias
            nc.sync.dma_start(bias_f[:], bfc_d[:])
            rc = [cst.tile([1, CH], bf16, name=f"recip{h}")
                  for h in range(2 * HPC)]

            # qkv weights: 8 d-tiles of [128, 384] = [q128 | k128 | v128]
            wq = cst.tile([128, 8 * 384], bf16)
            for d in range(8):
                nc.sync.dma_start(wq[:, d * 384:(d + 1) * 384],
                                  wqkv[d * 128:(d + 1) * 128, :])
            # fc weights: 8 d-tiles of [128, 128]
            wfc = cst.tile([128, 8 * 128], bf16)
            for d in range(8):
                nc.sync.dma_start(wfc[:, d * 128:(d + 1) * 128],
                                  wfc_d[d * 128:(d + 1) * 128, :])

            # zero the rotating score-PSUM buffers once: diagonal score
            # matmuls leave stale left columns that exp reads before the
            # mask zeroes them -- first use must not see NaN bit patterns
            for zi in range(2):
                stz = ps.tile([128, 2 * CH], f32, tag="s", bufs=2,
                              name=f"stz{zi}")
                nc.scalar.memzero(stz[:])

            def make_tiles(b, rep):
                qt = wk.tile([128, T], bf16, tag="qt", bufs=2,
                             name=f"qt{b}_{rep}")
                kt = wk.tile([128, T], bf16, tag="kt", bufs=2,
                             name=f"kt{b}_{rep}")
                vsb = wk.tile([128, NKV_B * 130], bf16, tag="vsb", bufs=2,
                              name=f"vsb{b}_{rep}")
                # ones columns (softmax denominator) for all 16 kv tiles
                v3 = vsb.rearrange("p (t c) -> p t c", c=130)
                src1 = onesh[:, 0:NKV_B].rearrange("p (t c) -> p t c", c=1)
                nc.sync.dma_start(v3[:, :, 64:65], src1)
                nc.sync.dma_start(v3[:, :, 129:130], src1)
                return qt, kt, vsb

            def qkv_steps(b, rep, tiles):
                """Generator: one yield per PE step of batch b's projection."""
                qt, kt, vsb = tiles
                xts = {}

                def xt_dma(ch):
                    xt = wk.tile([128, 8 * CH], bf16, tag="xt", bufs=2,
                                 name=f"xt{b}_{ch}_{rep}")
                    nc.sync.dma_start(
                        xt.rearrange("p (d c) -> p d c", d=8),
                        xT[:, b * T + ch * CH:b * T + (ch + 1) * CH]
                        .rearrange("(d p) c -> p d c", p=128))
                    xts[ch] = xt

                xt_dma(0)
                for ch in range(NCH_B):
                    if ch + 1 < NCH_B:
                        xt_dma(ch + 1)
                    xt = xts.pop(ch)
                    xt3 = xt.rearrange("p (d c) -> p d c", d=8)
                    cs = ch * CH
                    # Q^T chunk (bias added per-partition on PSUM drain)
                    psq = ps.tile([128, CH], f32, tag="mm", bufs=2,
                                  name=f"psq{b}_{ch}_{rep}")
                    for d in range(8):
                        nc.tensor.matmul(psq[:], wq[:, d * 384:d * 384 + 128],
                                         xt[:, d * CH:(d + 1) * CH],
                                         start=(d == 0), stop=(d == 7))
                        yield
                    nc.vector.tensor_scalar_add(qt[:, cs:cs + CH], psq[:],
                                                bias_q[:])
                    # K^T chunk (bias dropped: softmax-invariant)
                    psk = ps.tile([128, CH], f32, tag="mm", bufs=2,
                                  name=f"psk{b}_{ch}_{rep}")
                    for d in range(8):
                        nc.tensor.matmul(psk[:],
                                         wq[:, d * 384 + 128:d * 384 + 256],
                                         xt[:, d * CH:(d + 1) * CH],
                                         start=(d == 0), stop=(d == 7))
                        yield
                    nc.vector.tensor_copy(kt[:, cs:cs + CH], psk[:])
                    # V token-major (bias folded into FC bias on host)
                    for sb in range(CH // 128):
                        kvt = ch * 4 + sb
                        psv = ps.tile([128, 128], f32, tag="mm", bufs=2,
                                      name=f"psv{b}_{ch}_{sb}_{rep}")
                        for d in range(8):
                            nc.tensor.matmul(
                                psv[:], xt3[:, d, sb * 128:(sb + 1) * 128],
                                wq[:, d * 384 + 256:d * 384 + 384],
                                start=(d == 0), stop=(d == 7))
                        yield
                        # both heads' 64 V columns in one strided copy,
                        # skipping the interleaved ones columns
                        v4 = vsb.rearrange("p (t g c) -> p t g c", g=2, c=65)
                        nc.vector.tensor_copy(
                            v4[:, kvt, :, 0:64],
                            psv.rearrange("p (g c) -> p g c", c=64))

            def fc_steps(b, rep, ag_out):
                """Generator: one yield per PE step of batch b's FC layer."""
                ost = wk.tile([128, T], f32, tag="ost", bufs=2,
                              name=f"ost{b}_{rep}")
                fcis = {}

                def fci_dma(lc):
                    fci = wk.tile([128, 8 * CH], bf16, tag="fci", bufs=3,
                                  name=f"fci{b}_{lc}_{rep}")
                    nc.scalar.dma_start(
                        fci.rearrange("p (d c) -> p d c", d=8),
                        ag_out[:, lc * CH:(lc + 1) * CH]
                        .rearrange("(d p) c -> p d c", p=128))
                    fcis[lc] = fci

                fci_dma(0)
                for lc in range(QC):
                    if lc + 1 < QC:
                        fci_dma(lc + 1)
                    fci = fcis.pop(lc)
                    oc = b * QC + lc
                    pfc = ps.tile([128, CH], f32, tag="mm", bufs=2,
                                  name=f"pfc{oc}_{rep}")
                    for d in range(8):
                        nc.tensor.matmul(pfc[:], wfc[:, d * 128:(d + 1) * 128],
                                         fci[:, d * CH:(d + 1) * CH],
                                         start=(d == 0), stop=(d == 7))
                        yield
                    nc.vector.tensor_scalar_add(ost[:, lc * CH:(lc + 1) * CH],
                                                pfc[:], bias_f[:])
                nc.sync.dma_start(outT[:, b * T:(b + 1) * T], ost[:])

            def attn_batch(b, rep, tiles, filler):
                """Causal attention for batch b; pulls filler PE steps
                between P@V matmuls to hide the score->exp->mask latency."""
                qt, kt, vsb = tiles
                attn_b = wk.tile([128, T], bf16, tag="attn", bufs=2,
                                 name=f"attn{b}_{rep}") if strip >= 5 else None

                def pull(n):
                    for _ in range(n):
                        if next(filler, None) is None:
                            break

                for qc in range(QC):
                    nkv = 4 * (qc + 1)
                    pv = [ps.tile([128, CH], f32, tag=f"pv{h}", bufs=1,
                                  name=f"pv{h}_{b}_{qc}_{rep}")
                          for h in range(HPC)] if strip >= 4 else None

                    def scores(ki):
                        diag = ki - 4 * qc  # >=0 on diagonal block tiles
                        lo = 128 * diag if diag > 0 else 0
                        st = ps.tile([128, 2 * CH], f32, tag="s", bufs=2,
                                     name=f"s_{b}_{qc}_{ki}_{rep}")
                        for h in range(HPC):
                            nc.tensor.matmul(
                                st[:, h * CH + lo:(h + 1) * CH],
                                kt[64 * h:64 * h + 64,
                                   ki * 128:(ki + 1) * 128],
                                qt[64 * h:64 * h + 64,
                                   qc * CH + lo:(qc + 1) * CH],
                                start=True, stop=True,
                                tile_position=(64 * h, 0))
                        if strip < 2:
                            return None
                        pt = wk.tile([128, 2 * CH], bf16, tag="p", bufs=4,
                                     name=f"p_{b}_{qc}_{ki}_{rep}")
                        nc.scalar.activation(pt[:], st[:], EXP, scale=SCALE)
                        if diag >= 0 and strip >= 3:
                            off = 384 - 128 * diag
                            for h in range(HPC):
                                nc.vector.tensor_mul(
                                    pt[:, h * CH:(h + 1) * CH],
                                    pt[:, h * CH:(h + 1) * CH],
                                    hm[:, off:off + CH])
                        return pt

                    # scores run one kv tile ahead of P@V
                    pts = {0: scores(0)}
                    for ki in range(nkv):
                        if ki + 1 < nkv:
                            pts[ki + 1] = scores(ki + 1)
                        pull(fill_per_ki * (3 if ki == 0 else 1))
                        pt = pts.pop(ki)
                        if strip < 4:
                            continue
                        for h in range(HPC):
                            vb = ki * 130 + 65 * h
                            nc.tensor.matmul(pv[h][0:65, :],
                                             vsb[:, vb:vb + 65],
                                             pt[:, h * CH:(h + 1) * CH],
                                             start=(ki == 0),
                                             stop=(ki == nkv - 1))
                    if strip < 5:
                        continue
                    # normalize: reciprocal of denom row, broadcast via K=1
                    # ones-row matmul, multiply into attn_b (bf16); the two
                    # heads' chains are interleaved so they pipeline
                    par = HPC * (qc % 2)
                    with nc.allow_low_precision(
                            reason="softmax reciprocal broadcast in bf16; "
                                   "0.4% rounding is within tolerance"):
                        for h in range(HPC):
                            nc.vector.reciprocal(rc[par + h][:],
                                                 pv[h][64:65, :])
                    rbs = []
                    for h in range(HPC):
                        bc = ps.tile([64, CH], f32, tag="mm", bufs=2,
                                     name=f"bc{h}_{b}_{qc}_{rep}")
                        nc.tensor.matmul(bc[:], onesh[0:1, 0:64],
                                         rc[par + h][:],
                                         start=True, stop=True)
                        rb = wk.tile([64, CH], f32, tag="rb", bufs=2,
                                     name=f"rb{h}_{b}_{qc}_{rep}")
                        nc.vector.tensor_copy(rb[:], bc[:])
                        rbs.append(rb)
                    for h in range(HPC):
                        nc.vector.tensor_mul(
                            attn_b[64 * h:64 * h + 64,
                                   qc * CH:(qc + 1) * CH],
                            pv[h][0:64, :], rbs[h][:])
                # drain whatever filler the attention loop did not consume
                pull(1 << 20)
                return attn_b

            def ag_batch(b, rep, attn_b):
                ag_in = dpool.tile([128, T], bf16, name=f"ag_in{b}_{rep}")
                ag_out = dpool.tile([NCORES * 128, T], bf16,
                                    name=f"ag_out{b}_{rep}",
                                    addr_space="Shared")
                nc.sync.dma_start(ag_in[:], attn_b[:])
                if sim or no_collective:
                    nc.sync.dma_start(ag_out[0:128, :], ag_in[:])
                else:
                    nc.gpsimd.collective_compute(
                        "AllGather", mybir.AluOpType.bypass,
                        replica_groups=[list(range(NCORES))],
                        ins=[ag_in.opt()], outs=[ag_out.opt()])
                    for _agi in range(1, n_ag):   # timing-ablation only
                        ag_x = dpool.tile([NCORES * 128, T], bf16,
                                          name=f"ag_x{b}_{rep}_{_agi}",
                                          addr_space="Shared")
                        nc.gpsimd.collective_compute(
                            "AllGather", mybir.AluOpType.bypass,
                            replica_groups=[list(range(NCORES))],
                            ins=[ag_in.opt()], outs=[ag_x.opt()])
                return ag_out

            for rep in range(reps):
                tiles = make_tiles(0, rep)
                for _ in qkv_steps(0, rep, tiles):
                    pass
                batch_tiles = {0: tiles}
                ag_outs = {}
                for b in range(B):
                    fill = []
                    if strip >= 1 and b + 1 < B:
                        batch_tiles[b + 1] = make_tiles(b + 1, rep)
                        fill.append(qkv_steps(b + 1, rep, batch_tiles[b + 1]))
                    if b - 1 in ag_outs:
                        fill.append(fc_steps(b - 1, rep, ag_outs.pop(b - 1)))
                    filler = itertools.chain(*fill)
                    if strip >= 1:
                        attn_b = attn_batch(b, rep, batch_tiles.pop(b), filler)
                    else:
                        attn_b = None
                    for _ in filler:   # force-drain leftover filler
                        pass
                    if strip >= 6:
                        ag_outs[b] = ag_batch(b, rep, attn_b)
                if B - 1 in ag_outs:
                    for _ in fc_steps(B - 1, rep, ag_outs.pop(B - 1)):
                        pass

    nc.compile()
    return nc


def _host_inputs(x, W_qkv, b_qkv, W_fc, b_fc):
    import ml_dtypes
    x = np.asarray(x, dtype=np.float32)
    W_qkv = np.asarray(W_qkv, dtype=np.float32)
    b_qkv = np.asarray(b_qkv, dtype=np.float32)
    W_fc = np.asarray(W_fc, dtype=np.float32)
    b_fc = np.asarray(b_fc, dtype=np.float32)

    xT = np.ascontiguousarray(x.reshape(BT, D).T).astype(ml_dtypes.bfloat16)
    hm = (np.arange(128)[:, None]
          <= np.arange(896)[None, :] - 384).astype(ml_dtypes.bfloat16)
    ones = np.ones((128, CH), np.float32)
    onesh = np.ones((128, 64), ml_dtypes.bfloat16)
    b_v = b_qkv[2 * D:3 * D]
    bfc_eff = b_fc + b_v @ W_fc      # V bias folded through the FC layer
    in_maps = []
    for c in range(NCORES):
        f0 = c * (HPC * HD)  # 128*c
        wfc_c = np.ascontiguousarray(W_fc[:, f0:f0 + 128]).astype(
            ml_dtypes.bfloat16)
        bfc_c = np.ascontiguousarray(bfc_eff[f0:f0 + 128, None])
        wq_c = np.ascontiguousarray(np.concatenate(
            [W_qkv[:, p * D + f0: p * D + f0 + 128] for p in range(3)],
            axis=1).astype(ml_dtypes.bfloat16))
        bq_c = np.ascontiguousarray(b_qkv[f0:f0 + 128, None])
        in_maps.append({
            "xT": xT, "wqkv": wq_c, "bq": bq_c, "wfc": wfc_c, "bfc": bfc_c,
            "hm": hm, "ones": ones, "onesh": onesh,
        })
    return in_maps


def _get_nc():
    if "nc" not in _CACHE:
        _CACHE["nc"] = _build()
    return _CACHE["nc"]


def _assemble(results):
    blocks = [results[c]["outT"] for c in range(NCORES)]
    full = np.concatenate(blocks, axis=0)          # [D, BT], feature-major
    return np.ascontiguousarray(full.T).reshape(B, T, D).astype(np.float32)


def kernel(x, W_qkv, b_qkv, W_fc, b_fc):
    nc = _get_nc()
    in_maps = _host_inputs(x, W_qkv, b_qkv, W_fc, b_fc)
    res = run_bass_kernel_spmd(nc, in_maps, list(range(NCORES)))
    return _assemble(res.results)


# revision 47
# speedup vs baseline: 14121.7339x; 1.0301x over previous
"""Trainium2 Bass kernel for causal multi-head attention (B=4, T=2048, D=1024, H=16).

Sharding: tensor-parallel over heads. Each of the 8 NeuronCores owns 2 heads:
it computes Q/K/V projections for its head-slice over all tokens, runs causal
attention, then an AllGather re-shards the attention output from head-sharded
to token-major-full so each core computes the final FC layer for its 128
output features over all tokens. No reduction collective is needed.

v7 structure: software-pipelined at two levels. Across batches, batch b's
attention runs while batch b+1's QKV projection and batch b-1's FC layer
execute -- their matmuls are interleaved as dependency-free PE "filler"
between the P@V matmuls of the attention inner loop, so the PE never idles
waiting out the score -> exp -> mask chain (the attention-layer dependency
latency is ~1.5us per kv tile but its PE work is only ~0.85us). Within the
inner loop, scores run one kv tile ahead of P@V.

Numerics: Q/K/V, P = exp(S), the attention output, AllGather payload, and
FC input are bf16; matmul accumulation is fp32 in PSUM. The K projection
bias is dropped (it shifts every score in a softmax row by a constant) and
the V bias is folded into the FC bias on the host (b_fc' = b_fc + b_v@W_fc).
Scores are computed transposed (S^T = K Q^T) so softmax normalization lands
on the PV matmul's free dim; the denominator comes from a ones column
appended to V, and its reciprocal is broadcast across partitions with a
K=1 ones-row matmul (partition-shifted DVE/DMA ops misbehave on this stack).
Q/FC biases are added by the DVE during the PSUM drain (per-partition
tensor_scalar); diagonal score matmuls skip fully-masked columns (the causal
mask multiply cleans the stale columns, zeroed once at startup).
"""
import sys
import itertools
from collections import deque

for _p in ("/opt/trn_rl_repo",):
    if _p not in sys.path:
        sys.path.insert(0, _p)

import numpy as np

import concourse.bass as bass
import concourse.mybir as mybir
import concourse.tile as tile
from concourse import bacc
from concourse.bass_utils import run_bass_kernel_spmd

f32 = mybir.dt.float32
f32r = mybir.dt.float32r
bf16 = mybir.dt.bfloat16
EXP = mybir.ActivationFunctionType.Exp

B, T, D, H, HD = 4, 2048, 1024, 16, 64
NCORES = 8
HPC = H // NCORES          # heads per core = 2
BT = B * T                 # 8192
CH = 512                   # token chunk (matmul moving dim)
NCH_B = T // CH            # 4 projection chunks per batch
QC = T // CH               # 4 query chunks per batch
NKV_B = T // 128           # 16 kv tiles of 128 per batch
SCALE = 1.0 / 8.0          # 1/sqrt(HD)

_CACHE = {}
_DONE = object()   # generator-exhaustion sentinel (steps yield None)


def _build(sim=False, no_collective=False, reps=1, n_ag=1, strip=6,
           fill_per_ki=2):
    # strip: timing-diagnostic ladder. 6=full kernel; 5=drop ag/fc;
    # 4=also drop normalize; 3=also drop pv; 2=also drop causal mask;
    # 1=also drop exp; 0=qkv only.
    nc = bacc.Bacc("TRN2", target_bir_lowering=False, debug=False,
                   num_devices=1 if sim else NCORES)

    xT = nc.dram_tensor("xT", [D, BT], bf16, kind="ExternalInput").ap()
    wqkv = nc.dram_tensor("wqkv", [D, 3 * 128], bf16, kind="ExternalInput").ap()
    bq_d = nc.dram_tensor("bq", [128, 1], f32, kind="ExternalInput").ap()
    wfc_d = nc.dram_tensor("wfc", [D, 128], bf16, kind="ExternalInput").ap()
    bfc_d = nc.dram_tensor("bfc", [128, 1], f32, kind="ExternalInput").ap()
    hm_d = nc.dram_tensor("hm", [128, 896], bf16, kind="ExternalInput").ap()
    ones_d = nc.dram_tensor("ones", [128, CH], f32, kind="ExternalInput").ap()
    onesh_d = nc.dram_tensor("onesh", [128, 64], bf16, kind="ExternalInput").ap()
    outT = nc.dram_tensor("outT", [128, BT], f32, kind="ExternalOutput").ap()

    with tile.TileContext(nc) as tc:
        with tc.tile_pool(name="const", bufs=1) as cst, \
             tc.tile_pool(name="dram", bufs=1, space="DRAM") as dpool, \
             tc.tile_pool(name="work", bufs=1) as wk, \
             tc.tile_pool(name="ps", bufs=1, space="PSUM") as ps:

            # ---- constants (host-provided) ----
            ones_r = cst.tile([1, CH], f32r)
            nc.sync.dma_start(ones_r[:], ones_d[0:1, :].bitcast(f32r))
            onesh = cst.tile([128, 64], bf16)
            nc.sync.dma_start(onesh[:], onesh_d[:])
            hm = cst.tile([128, 896], bf16)
            nc.sync.dma_start(hm[:], hm_d[:])
            bias_q = cst.tile([128, 1], f32)     # per-partition Q bias
            nc.sync.dma_start(bias_q[:], bq_d[:])
            bias_f = cst.tile([128, 1], f32)     # per-partition FC bias
            nc.sync.dma_start(bias_f[:], bfc_d[:])
            rc = [cst.tile([1, CH], bf16, name=f"recip{h}")
                  for h in range(2 * HPC)]

            # qkv weights: 8 d-tiles of [128, 384] = [q128 | k128 | v128]
            wq = cst.tile([128, 8 * 384], bf16)
            for d in range(8):
                nc.sync.dma_start(wq[:, d * 384:(d + 1) * 384],
                                  wqkv[d * 128:(d + 1) * 128, :])
            # fc weights: 8 d-tiles of [128, 128]
            wfc = cst.tile([128, 8 * 128], bf16)
            for d in range(8):
                nc.sync.dma_start(wfc[:, d * 128:(d + 1) * 128],
                                  wfc_d[d * 128:(d + 1) * 128, :])

            # zero the rotating score-PSUM buffers once: diagonal score
            # matmuls leave stale left columns that exp reads before the
            # mask zeroes them -- first use must not see NaN bit patterns
            for zi in range(2):
                stz = ps.tile([128, 2 * CH], f32, tag="s", bufs=2,
                              name=f"stz{zi}")
                nc.scalar.memzero(stz[:])

            def make_tiles(b, rep):
                qt = wk.tile([128, T], bf16, tag="qt", bufs=2,
                             name=f"qt{b}_{rep}")
                kt = wk.tile([128, T], bf16, tag="kt", bufs=2,
                             name=f"kt{b}_{rep}")
                vsb = wk.tile([128, NKV_B * 130], bf16, tag="vsb", bufs=2,
                              name=f"vsb{b}_{rep}")
                # ones columns (softmax denominator) for all 16 kv tiles
                v3 = vsb.rearrange("p (t c) -> p t c", c=130)
                src1 = onesh[:, 0:NKV_B].rearrange("p (t c) -> p t c", c=1)
                nc.sync.dma_start(v3[:, :, 64:65], src1)
                nc.sync.dma_start(v3[:, :, 129:130], src1)
                return qt, kt, vsb

            def qkv_steps(b, rep, tiles):
                """Generator: one yield per PE step of batch b's projection."""
                qt, kt, vsb = tiles
                xts = {}

                def xt_dma(ch):
                    xt = wk.tile([128, 8 * CH], bf16, tag="xt", bufs=2,
                                 name=f"xt{b}_{ch}_{rep}")
                    nc.sync.dma_start(
                        xt.rearrange("p (d c) -> p d c", d=8),
                        xT[:, b * T + ch * CH:b * T + (ch + 1) * CH]
                        .rearrange("(d p) c -> p d c", p=128))
                    xts[ch] = xt

                xt_dma(0)
                for ch in range(NCH_B):
                    if ch + 1 < NCH_B:
                        xt_dma(ch + 1)
                    xt = xts.pop(ch)
                    xt3 = xt.rearrange("p (d c) -> p d c", d=8)
                    cs = ch * CH
                    # Q^T chunk (bias added per-partition on PSUM drain)
                    psq = ps.tile([128, CH], f32, tag="mm", bufs=2,
                                  name=f"psq{b}_{ch}_{rep}")
                    for d in range(8):
                        nc.tensor.matmul(psq[:], wq[:, d * 384:d * 384 + 128],
                                         xt[:, d * CH:(d + 1) * CH],
                                         start=(d == 0), stop=(d == 7))
                        yield
                    nc.vector.tensor_scalar_add(qt[:, cs:cs + CH], psq[:],
                                                bias_q[:])
                    # K^T chunk (bias dropped: softmax-invariant)
                    psk = ps.tile([128, CH], f32, tag="mm", bufs=2,
                                  name=f"psk{b}_{ch}_{rep}")
                    for d in range(8):
                        nc.tensor.matmul(psk[:],
                                         wq[:, d * 384 + 128:d * 384 + 256],
                                         xt[:, d * CH:(d + 1) * CH],
                                         start=(d == 0), stop=(d == 7))
                        yield
                    nc.vector.tensor_copy(kt[:, cs:cs + CH], psk[:])
                    # V token-major (bias folded into FC bias on host)
                    for sb in range(CH // 128):
                        kvt = ch * 4 + sb
                        psv = ps.tile([128, 128], f32, tag="mm", bufs=2,
                                      name=f"psv{b}_{ch}_{sb}_{rep}")
                        for d in range(8):
                            nc.tensor.matmul(
                                psv[:], xt3[:, d, sb * 128:(sb + 1) * 128],
                                wq[:, d * 384 + 256:d * 384 + 384],
                                start=(d == 0), stop=(d == 7))
                        yield
                        # both heads' 64 V columns in one strided copy,
                        # skipping the interleaved ones columns
                        v4 = vsb.rearrange("p (t g c) -> p t g c", g=2, c=65)
                        nc.vector.tensor_copy(
                            v4[:, kvt, :, 0:64],
                            psv.rearrange("p (g c) -> p g c", c=64))

            def fc_steps(b, rep, ag_out, half):
                """Generator: one yield per PE step of half of batch b's FC
                layer (ag_out covers token columns [half*T/2, (half+1)*T/2))."""
                HT = T // 2
                ost = wk.tile([128, HT], f32, tag="ost", bufs=2,
                              name=f"ost{b}_{half}_{rep}")
                lcs = (0, 1) if half == 0 else (2, 3)
                fcis = {}

                def fci_dma(lc):
                    fci = wk.tile([128, 8 * CH], bf16, tag="fci", bufs=3,
                                  name=f"fci{b}_{lc}_{rep}")
                    c0 = (lc - 2 * half) * CH
                    nc.scalar.dma_start(
                        fci.rearrange("p (d c) -> p d c", d=8),
                        ag_out[:, c0:c0 + CH]
                        .rearrange("(d p) c -> p d c", p=128))
                    fcis[lc] = fci

                fci_dma(lcs[0])
                for li, lc in enumerate(lcs):
                    if li + 1 < len(lcs):
                        fci_dma(lcs[li + 1])
                    fci = fcis.pop(lc)
                    oc = b * QC + lc
                    pfc = ps.tile([128, CH], f32, tag="mm", bufs=2,
                                  name=f"pfc{oc}_{rep}")
                    for d in range(8):
                        nc.tensor.matmul(pfc[:], wfc[:, d * 128:(d + 1) * 128],
                                         fci[:, d * CH:(d + 1) * CH],
                                         start=(d == 0), stop=(d == 7))
                        yield
                    c0 = (lc - 2 * half) * CH
                    nc.vector.tensor_scalar_add(ost[:, c0:c0 + CH],
                                                pfc[:], bias_f[:])
                nc.sync.dma_start(
                    outT[:, b * T + half * HT:b * T + (half + 1) * HT],
                    ost[:])

            def attn_batch(b, rep, tiles, fill_q, on_qc=None):
                """Causal attention for batch b; pulls filler PE steps from
                the fill_q deque between P@V matmuls to hide the
                score->exp->mask latency. on_qc(qc, attn_b) fires after each
                query chunk's normalize is emitted (used to issue half-batch
                AllGathers and append their FC work to fill_q)."""
                qt, kt, vsb = tiles
                attn_b = wk.tile([128, T], bf16, tag="attn", bufs=2,
                                 name=f"attn{b}_{rep}") if strip >= 5 else None

                def pull(n):
                    # generators yield None per PE step, so exhaustion must
                    # be detected with a sentinel, not None
                    while n > 0 and fill_q:
                        if next(fill_q[0], _DONE) is _DONE:
                            fill_q.popleft()
                        else:
                            n -= 1

                for qc in range(QC):
                    nkv = 4 * (qc + 1)
                    pv = [ps.tile([128, CH], f32, tag=f"pv{h}", bufs=1,
                                  name=f"pv{h}_{b}_{qc}_{rep}")
                          for h in range(HPC)] if strip >= 4 else None

                    def scores(ki):
                        diag = ki - 4 * qc  # >=0 on diagonal block tiles
                        lo = 128 * diag if diag > 0 else 0
                        st = ps.tile([128, 2 * CH], f32, tag="s", bufs=2,
                                     name=f"s_{b}_{qc}_{ki}_{rep}")
                        for h in range(HPC):
                            nc.tensor.matmul(
                                st[:, h * CH + lo:(h + 1) * CH],
                                kt[64 * h:64 * h + 64,
                                   ki * 128:(ki + 1) * 128],
                                qt[64 * h:64 * h + 64,
                                   qc * CH + lo:(qc + 1) * CH],
                                start=True, stop=True,
                                tile_position=(64 * h, 0))
                        if strip < 2:
                            return None
                        pt = wk.tile([128, 2 * CH], bf16, tag="p", bufs=4,
                                     name=f"p_{b}_{qc}_{ki}_{rep}")
                        nc.scalar.activation(pt[:], st[:], EXP, scale=SCALE)
                        if diag >= 0 and strip >= 3:
                            off = 384 - 128 * diag
                            for h in range(HPC):
                                nc.vector.tensor_mul(
                                    pt[:, h * CH:(h + 1) * CH],
                                    pt[:, h * CH:(h + 1) * CH],
                                    hm[:, off:off + CH])
                        return pt

                    # scores run one kv tile ahead of P@V
                    pts = {0: scores(0)}
                    for ki in range(nkv):
                        if ki + 1 < nkv:
                            pts[ki + 1] = scores(ki + 1)
                        pull(fill_per_ki * (3 if ki == 0 else 1))
                        pt = pts.pop(ki)
                        if strip < 4:
                            continue
                        for h in range(HPC):
                            vb = ki * 130 + 65 * h
                            nc.tensor.matmul(pv[h][0:65, :],
                                             vsb[:, vb:vb + 65],
                                             pt[:, h * CH:(h + 1) * CH],
                                             start=(ki == 0),
                                             stop=(ki == nkv - 1))
                    if strip < 5:
                        continue
                    # normalize: reciprocal of denom row, broadcast via K=1
                    # ones-row matmul, multiply into attn_b (bf16); the two
                    # heads' chains are interleaved so they pipeline
                    par = HPC * (qc % 2)
                    with nc.allow_low_precision(
                            reason="softmax reciprocal broadcast in bf16; "
                                   "0.4% rounding is within tolerance"):
                        for h in range(HPC):
                            nc.vector.reciprocal(rc[par + h][:],
                                                 pv[h][64:65, :])
                    rbs = []
                    for h in range(HPC):
                        bc = ps.tile([64, CH], f32, tag="mm", bufs=2,
                                     name=f"bc{h}_{b}_{qc}_{rep}")
                        nc.tensor.matmul(bc[:], onesh[0:1, 0:64],
                                         rc[par + h][:],
                                         start=True, stop=True)
                        rb = wk.tile([64, CH], f32, tag="rb", bufs=2,
                                     name=f"rb{h}_{b}_{qc}_{rep}")
                        nc.vector.tensor_copy(rb[:], bc[:])
                        rbs.append(rb)
                    for h in range(HPC):
                        nc.vector.tensor_mul(
                            attn_b[64 * h:64 * h + 64,
                                   qc * CH:(qc + 1) * CH],
                            pv[h][0:64, :], rbs[h][:])
                    if on_qc is not None:
                        on_qc(qc, attn_b)
                # drain whatever filler the attention loop did not consume
                pull(1 << 20)
                return attn_b

            def ag_half(b, rep, attn_b, half):
                """AllGather one half (1024 tokens) of batch b's attention;
                half 0 is issued mid-attention, right after qc1 completes."""
                HT = T // 2
                ag_in = dpool.tile([128, HT], bf16,
                                   name=f"ag_in{b}_{half}_{rep}")
                ag_out = dpool.tile([NCORES * 128, HT], bf16,
                                    name=f"ag_out{b}_{half}_{rep}",
                                    addr_space="Shared")
                nc.sync.dma_start(ag_in[:],
                                  attn_b[:, half * HT:(half + 1) * HT])
                if sim or no_collective:
                    nc.sync.dma_start(ag_out[0:128, :], ag_in[:])
                else:
                    nc.gpsimd.collective_compute(
                        "AllGather", mybir.AluOpType.bypass,
                        replica_groups=[list(range(NCORES))],
                        ins=[ag_in.opt()], outs=[ag_out.opt()])
                    for _agi in range(1, n_ag):   # timing-ablation only
                        ag_x = dpool.tile([NCORES * 128, HT], bf16,
                                          name=f"ag_x{b}_{half}_{rep}_{_agi}",
                                          addr_space="Shared")
                        nc.gpsimd.collective_compute(
                            "AllGather", mybir.AluOpType.bypass,
                            replica_groups=[list(range(NCORES))],
                            ins=[ag_in.opt()], outs=[ag_x.opt()])
                return ag_out

            for rep in range(reps):
                tiles = make_tiles(0, rep)
                for _ in qkv_steps(0, rep, tiles):
                    pass
                batch_tiles = {0: tiles}
                fc_pending = {}   # batch -> [fc_steps gens] for next batch
                half0 = {}        # batch -> half-0 ag_out
                for b in range(B):
                    fill_q = deque()
                    if strip >= 1 and b + 1 < B:
                        batch_tiles[b + 1] = make_tiles(b + 1, rep)
                        fill_q.append(qkv_steps(b + 1, rep,
                                                batch_tiles[b + 1]))
                    for g in fc_pending.pop(b - 1, ()):
                        fill_q.append(g)

                    def on_qc(qc, attn_b, b=b, fill_q=fill_q):
                        if strip < 6 or qc != 1:
                            return
                        half0[b] = ag_half(b, rep, attn_b, 0)
                        if b == B - 1:
                            # last batch: its first FC half becomes filler
                            # for the remaining attention chunks
                            fill_q.append(fc_steps(b, rep, half0[b], 0))

                    if strip >= 1:
                        attn_b = attn_batch(b, rep, batch_tiles.pop(b),
                                            fill_q, on_qc)
                    else:
                        attn_b = None
                    while fill_q:   # force-drain leftover filler
                        if next(fill_q[0], _DONE) is _DONE:
                            fill_q.popleft()
                    if strip >= 6:
                        ag1 = ag_half(b, rep, attn_b, 1)
                        if b < B - 1:
                            fc_pending[b] = [
                                fc_steps(b, rep, half0[b], 0),
                                fc_steps(b, rep, ag1, 1),
                            ]
                        else:
                            for _ in fc_steps(b, rep, ag1, 1):
                                pass

    nc.compile()
    return nc


def _host_inputs(x, W_qkv, b_qkv, W_fc, b_fc):
    import ml_dtypes
    x = np.asarray(x, dtype=np.float32)
    W_qkv = np.asarray(W_qkv, dtype=np.float32)
    b_qkv = np.asarray(b_qkv, dtype=np.float32)
    W_fc = np.asarray(W_fc, dtype=np.float32)
    b_fc = np.asarray(b_fc, dtype=np.float32)

    xT = np.ascontiguousarray(x.reshape(BT, D).T).astype(ml_dtypes.bfloat16)
    hm = (np.arange(128)[:, None]
          <= np.arange(896)[None, :] - 384).astype(ml_dtypes.bfloat16)
    ones = np.ones((128, CH), np.float32)
    onesh = np.ones((128, 64), ml_dtypes.bfloat16)
    b_v = b_qkv[2 * D:3 * D]
    bfc_eff = b_fc + b_v @ W_fc      # V bias folded through the FC layer
    in_maps = []
    for c in range(NCORES):
        f0 = c * (HPC * HD)  # 128*c
        wfc_c = np.ascontiguousarray(W_fc[:, f0:f0 + 128]).astype(
            ml_dtypes.bfloat16)
        bfc_c = np.ascontiguousarray(bfc_eff[f0:f0 + 128, None])
        wq_c = np.ascontiguousarray(np.concatenate(
            [W_qkv[:, p * D + f0: p * D + f0 + 128] for p in range(3)],
            axis=1).astype(ml_dtypes.bfloat16))
        bq_c = np.ascontiguousarray(b_qkv[f0:f0 + 128, None])
        in_maps.append({
            "xT": xT, "wqkv": wq_c, "bq": bq_c, "wfc": wfc_c, "bfc": bfc_c,
            "hm": hm, "ones": ones, "onesh": onesh,
        })
    return in_maps


def _get_nc():
    if "nc" not in _CACHE:
        _CACHE["nc"] = _build()
    return _CACHE["nc"]


def _assemble(results):
    blocks = [results[c]["outT"] for c in range(NCORES)]
    full = np.concatenate(blocks, axis=0)          # [D, BT], feature-major
    return np.ascontiguousarray(full.T).reshape(B, T, D).astype(np.float32)


def kernel(x, W_qkv, b_qkv, W_fc, b_fc):
    nc = _get_nc()
    in_maps = _host_inputs(x, W_qkv, b_qkv, W_fc, b_fc)
    res = run_bass_kernel_spmd(nc, in_maps, list(range(NCORES)))
    return _assemble(res.results)
